# revision 19
# baseline (speedup 1.0000x reference)
"""DCBlock kernel — full device pipeline on 8 NeuronCores, H-sharded.

Math (folding): BN is affine xn = a*x+b; 1x1 convs commute with the
affinity-weighted 7x7 aggregation (it is linear and per-pixel over space,
convs act on channels). With softmax weights summing to 1:
    out = x + W2a @ aggregate(x, affinity) + cvec
where W2a = (w_fuse @ w_feat) * a[col], cvec = (w_fuse @ w_feat) @ b.

Affinity: dist[k,p] = |cp[:,q_k]-cp[:,p]|^2 over 19 channels; z=exp(-d/den);
aff = softmax_k(exp(z)) (double exp per reference).

Device pipeline per core (strip = 16 rows x 86 padded cols, 10 output rows):
  1. G'[p, q-window] = (2*dot - sqn_p - sqn_q)/den via ONE K=57 matmul
     (rows 0-18 cp-center*2/den x cp-strip; 19-37 ones x (-cp^2/den);
      38-56 (-cpc^2/den) x ones).
  2. z = exp(G'), E = exp(z) on ACT engine (full window; band mask later).
  3. V = E * bandmask (bf16), S = row-sum(V) via accum_out; 1/S on DVE.
  4. Band aggregation: DMA-transpose V 128-col slices -> lhsT tiles;
     aggT[p,c] = sum_s V_s^T @ xT[walign+128s] (PSUM accumulated, bf16).
  5. aggT * (1/S) -> bf16, PE-transpose -> agg[c,p].
  6. refined = W2aT^T @ agg (bf16); out = refined + cvec + x (fused DVE op).

Pixel zero-padding on the 86-grid (host-padded xT and cp) makes all
out-of-image taps contribute exactly 0 while still occupying softmax mass,
matching the reference.
"""
import time
import numpy as np
import ml_dtypes

BF16 = ml_dtypes.bfloat16

K = 7
PAD = 3
B, C, H, W = 1, 256, 80, 80
CP = 19
NCORES = 8
RPC = H // NCORES            # 10 out rows per core
SR = RPC + 2 * PAD           # 16 strip rows
G86 = W + 2 * PAD            # 86 padded cols
NS86 = SR * G86              # 1376 strip positions (86-grid)
NSPAD = 1664                 # padded to 13 x 128
NPX = RPC * W                # 800 out px per core
PT = 7                       # ceil(800/128) pixel tiles
WIN = 896                    # G'/mask window (7 x 128)
BN_EPS = 1e-5

LAST_EXEC_NS = None


def _walign(t):
    """128-aligned start (on the padded 86-grid strip axis) of ptile t's
    7x7 neighborhood window."""
    p0 = 128 * t
    r0 = p0 // W
    w86 = G86 * r0
    return 128 * (w86 // 128)


def _build_masks():
    """masks[t][i, j] = 1 where j = q86 - walign(t) is a tap position of
    out pixel p = 128t + i. Rows with p >= NPX get a single dummy tap to
    keep S > 0 (avoids inf/NaN in unused lanes)."""
    masks = np.zeros((PT, 128, WIN), np.float32)
    for t in range(PT):
        wa = _walign(t)
        for i in range(128):
            p = 128 * t + i
            if p >= NPX:
                masks[t, i, 0] = 1.0
                continue
            r, c = divmod(p, W)
            sr = r + PAD
            for di in range(-PAD, PAD + 1):
                for dj in range(-PAD, PAD + 1):
                    q86 = (sr + di) * G86 + (c + PAD + dj)
                    j = q86 - wa
                    assert 0 <= j < WIN, (t, i, di, dj, j)
                    masks[t, i, j] = 1.0
    return masks.astype(BF16)


_MASKS = None


def _host_prep(x, coarse_probs, sigma, w_feat, w_fuse, bn_gamma, bn_beta,
               bn_mean, bn_var):
    global _MASKS
    a = (bn_gamma / np.sqrt(bn_var + BN_EPS)).astype(np.float32)
    b = (bn_beta - bn_mean * a).astype(np.float32)
    W2 = (w_fuse @ w_feat).astype(np.float32)
    w2at = np.ascontiguousarray((W2 * a[None, :]).T).astype(BF16)  # [cin,cout]
    cvec = (W2 @ b).astype(np.float32).reshape(C, 1)

    xp = np.zeros((C, G86, G86), np.float32)
    xp[:, PAD:PAD + H, PAD:PAD + W] = x[0]
    cpp = np.zeros((CP, G86, G86), np.float32)
    cpp[:, PAD:PAD + H, PAD:PAD + W] = coarse_probs[0]

    if _MASKS is None:
        _MASKS = _build_masks()
    masks2d = np.ascontiguousarray(_MASKS.reshape(PT * 128, WIN))

    ident = np.eye(128, dtype=BF16)
    den = 2.0 * max(float(sigma.reshape(-1)[0]), 0.0) ** 2 + 1e-8

    in_maps = []
    for core in range(NCORES):
        r0 = core * RPC
        xs = xp[:, r0:r0 + SR, :].reshape(C, NS86)
        xt = np.zeros((NSPAD, C), BF16)
        xt[:NS86] = xs.T.astype(BF16)
        cps = np.zeros((CP, NSPAD), np.float32)
        cps[:, :NS86] = cpp[:, r0:r0 + SR, :].reshape(CP, NS86)
        cpc = cpp[:, PAD + r0:PAD + r0 + RPC, PAD:PAD + W].reshape(CP, NPX)
        # Gram operands, K = 96 (32-aligned groups); G' = lhs.T @ rhs gives
        # z-arg = (2*dot - sqn_p - sqn_q)/den directly.
        grhs = np.zeros((96, NSPAD), np.float32)
        grhs[0:CP] = cps
        grhs[32:32 + CP] = -(cps * cps) / den
        grhs[64:64 + CP] = 1.0
        glhs = np.zeros((96, PT * 128), np.float32)
        glhs[0:CP, :NPX] = cpc * (2.0 / den)
        glhs[32:32 + CP, :NPX] = 1.0
        glhs[64:64 + CP, :NPX] = -(cpc * cpc) / den
        xc = np.ascontiguousarray(
            x[0, :, r0:r0 + RPC, :].reshape(C, NPX)).astype(BF16)
        gram = np.ascontiguousarray(np.concatenate([grhs, glhs], axis=1))
        in_maps.append({
            "xt": xt, "gram": gram, "xc": xc, "masks": masks2d,
            "w2at": w2at, "cvec": cvec, "ident": ident,
        })
    return in_maps


def _build_program():
    import concourse.bass as bass
    import concourse.mybir as mybir
    from concourse.tile import TileContext

    f32 = mybir.dt.float32
    f32r = mybir.dt.float32r
    bf16 = mybir.dt.bfloat16
    ALU = mybir.AluOpType
    ACTF = mybir.ActivationFunctionType

    nc = bass.Bass()
    xt_d = nc.dram_tensor("xt", [NSPAD, C], bf16, kind="ExternalInput")
    gram_d = nc.dram_tensor("gram", [96, NSPAD + PT * 128], f32r,
                            kind="ExternalInput")
    xc_d = nc.dram_tensor("xc", [C, NPX], bf16, kind="ExternalInput")
    masks_d = nc.dram_tensor("masks", [PT * 128, WIN], bf16,
                             kind="ExternalInput")
    w2at_d = nc.dram_tensor("w2at", [C, C], bf16, kind="ExternalInput")
    cvec_d = nc.dram_tensor("cvec", [C, 1], f32, kind="ExternalInput")
    ident_d = nc.dram_tensor("ident", [128, 128], bf16, kind="ExternalInput")
    out_d = nc.dram_tensor("out", [C, NPX], f32, kind="ExternalOutput")

    NT86 = NSPAD // 128      # 13 xT tiles

    with TileContext(nc) as tc:
        with tc.tile_pool(name="const", bufs=1) as cpool, \
             tc.tile_pool(name="work", bufs=2) as wpool, \
             tc.tile_pool(name="ps", bufs=2, space="PSUM") as ppool:

            # ---- constant loads ----
            xt_sb = []
            for a_i in range(NT86):
                t_ = cpool.tile([128, C], bf16, tag=f"xt{a_i}", name=f"xt{a_i}")
                nc.sync.dma_start(t_[:, :], xt_d[128 * a_i:128 * (a_i + 1), :])
                xt_sb.append(t_)
            mask_sb = []
            for t in range(PT):
                m_ = cpool.tile([128, WIN], bf16, tag=f"mk{t}", name=f"mk{t}")
                nc.sync.dma_start(m_[:, :], masks_d[128 * t:128 * (t + 1), :])
                mask_sb.append(m_)
            xc_sb = []
            w2_sb = []
            for h in range(2):
                xc_ = cpool.tile([128, NPX], bf16, tag=f"xc{h}", name=f"xc{h}")
                nc.sync.dma_start(xc_[:, :], xc_d[128 * h:128 * (h + 1), :])
                xc_sb.append(xc_)
                w2_ = cpool.tile([128, C], bf16, tag=f"w2{h}", name=f"w2{h}")
                nc.sync.dma_start(w2_[:, :], w2at_d[128 * h:128 * (h + 1), :])
                w2_sb.append(w2_)
            cv_sb = cpool.tile([128, 2], f32, tag="cv", name="cv")
            for h in range(2):
                nc.sync.dma_start(cv_sb[:, h:h + 1],
                                  cvec_d[128 * h:128 * (h + 1), :])
            id_sb = cpool.tile([128, 128], bf16, tag="id", name="id")
            nc.sync.dma_start(id_sb[:, :], ident_d[:, :])

            gram_t = cpool.tile([96, NSPAD + PT * 128], f32r, tag="gram",
                                name="gram_t")
            nc.sync.dma_start(gram_t[:, :], gram_d[:, :])
            rhs_t = gram_t[:, 0:NSPAD]
            lhs_t = gram_t[:, NSPAD:NSPAD + PT * 128]

            agg_sb = [cpool.tile([128, PT * 128], bf16, tag=f"agg{h}",
                                 name=f"agg{h}") for h in range(2)]

            # ---- per pixel-tile pipeline ----
            for t in range(PT):
                wa = _walign(t)
                gp = []
                for half in range(2):
                    g_ = ppool.tile([128, 448], f32, tag="gp", name=f"gp{t}_{half}")
                    nc.tensor.matmul(
                        g_[:, :],
                        lhsT=lhs_t[:, 128 * t:128 * (t + 1)],
                        rhs=rhs_t[:, wa + 448 * half:wa + 448 * (half + 1)],
                        start=True, stop=True)
                    gp.append(g_)
                z = wpool.tile([128, WIN], f32, tag="z", name=f"z{t}")
                for half in range(2):
                    nc.scalar.activation(z[:, 448 * half:448 * (half + 1)],
                                         gp[half][:, :], ACTF.Exp)
                e = wpool.tile([128, WIN], f32, tag="e", name=f"e{t}")
                nc.scalar.activation(e[:, :], z[:, :], ACTF.Exp)
                v = wpool.tile([128, WIN], bf16, tag="v", name=f"v{t}")
                s_acc = wpool.tile([128, 1], f32, tag="s", name=f"s{t}")
                nc.vector.scalar_tensor_tensor(
                    v[:, :], e[:, :], 1.0, mask_sb[t][:, :],
                    ALU.mult, ALU.mult, accum_out=s_acc[:, :])
                srec = wpool.tile([128, 1], f32, tag="sr", name=f"sr{t}")
                nc.vector.reciprocal(srec[:, :], s_acc[:, :])

                # band tiles: transpose each 128-col slice of V
                aggp = ppool.tile([128, C], f32, tag="aggp", name=f"aggp{t}")
                for s in range(PT):
                    vt = wpool.tile([128, 128], bf16, tag=f"vt{s % 2}",
                                    name=f"vt{t}_{s}")
                    nc.sync.dma_start_transpose(
                        vt[:, :], v[:, 128 * s:128 * (s + 1)])
                    nc.tensor.matmul(aggp[:, :], lhsT=vt[:, :],
                                     rhs=xt_sb[wa // 128 + s][:, :],
                                     start=(s == 0), stop=(s == PT - 1))
                aggd = wpool.tile([128, C], bf16, tag="aggd", name=f"aggd{t}")
                nc.vector.tensor_scalar(aggd[:, :], aggp[:, :], srec[:, :],
                                        None, ALU.mult)
                for h in range(2):
                    tp = ppool.tile([128, 128], bf16, tag="tp",
                                    name=f"tp{t}_{h}")
                    nc.tensor.transpose(tp[:, :],
                                        aggd[:, 128 * h:128 * (h + 1)],
                                        id_sb[:, :])
                    nc.scalar.copy(
                        agg_sb[h][:, 128 * t:128 * (t + 1)], tp[:, :])

            # ---- fuse conv + residual ----
            NSP = [(0, 400), (400, 400)]
            for h in range(2):
                for (ns, nn) in NSP:
                    fp = ppool.tile([128, 400], f32, tag="fp",
                                    name=f"fp{h}_{ns}")
                    for kc in range(2):
                        nc.tensor.matmul(
                            fp[:, :],
                            lhsT=w2_sb[kc][:, 128 * h:128 * (h + 1)],
                            rhs=agg_sb[kc][:, ns:ns + nn],
                            start=(kc == 0), stop=(kc == 1))
                    o_sb = wpool.tile([128, 400], f32, tag="o",
                                      name=f"o{h}_{ns}")
                    nc.vector.scalar_tensor_tensor(
                        o_sb[:, :], fp[:, :],
                        cv_sb[:, h:h + 1],
                        xc_sb[h][:, ns:ns + nn], ALU.add, ALU.add)
                    nc.sync.dma_start(
                        out_d[128 * h:128 * (h + 1), ns:ns + nn], o_sb[:, :])
    return nc


def _device_run(in_maps):
    global LAST_EXEC_NS
    from concourse.bass_utils import run_bass_kernel_spmd
    nc = _build_program()
    res = run_bass_kernel_spmd(nc, in_maps, list(range(NCORES)))
    t0 = time.perf_counter()
    res = run_bass_kernel_spmd(nc, in_maps, list(range(NCORES)))
    LAST_EXEC_NS = (time.perf_counter() - t0) * 1e9
    out = np.empty((B, C, H, W), np.float32)
    for i in range(NCORES):
        out[0, :, i * RPC:(i + 1) * RPC, :] = \
            np.asarray(res.results[i]["out"]).reshape(C, RPC, W)
    return out


def _host_fallback(x, coarse_probs, sigma, w_feat, w_fuse, bn_gamma, bn_beta,
                   bn_mean, bn_var):
    inv = 1.0 / np.sqrt(bn_var + BN_EPS)
    xn = ((x - bn_mean[None, :, None, None])
          * (inv * bn_gamma)[None, :, None, None]
          + bn_beta[None, :, None, None]).astype(np.float32)
    denom = 2.0 * max(float(sigma.reshape(-1)[0]), 0.0) ** 2 + 1e-8
    cpp = np.pad(coarse_probs, ((0, 0), (0, 0), (PAD, PAD), (PAD, PAD)))
    zs = np.empty((K * K, B, H, W), np.float32)
    for idx in range(K * K):
        i, j = divmod(idx, K)
        shifted = cpp[:, :, i:i + H, j:j + W]
        zs[idx] = np.exp(-np.sum((shifted - coarse_probs) ** 2, axis=1)
                         / denom)
    es = np.exp(zs)
    aff = es / es.sum(axis=0, keepdims=True)
    msgs = np.einsum('oc,bchw->bohw', w_feat, xn).astype(np.float32)
    mp = np.pad(msgs, ((0, 0), (0, 0), (PAD, PAD), (PAD, PAD)))
    agg = np.zeros((B, C, H, W), np.float32)
    for idx in range(K * K):
        i, j = divmod(idx, K)
        agg += mp[:, :, i:i + H, j:j + W] * aff[idx][:, None]
    refined = np.einsum('oc,bchw->bohw', w_fuse, agg).astype(np.float32)
    return (x + refined).astype(np.float32)


def kernel(x, coarse_probs, sigma, w_feat, w_fuse, bn_gamma, bn_beta,
           bn_mean, bn_var):
    x = np.asarray(x, np.float32)
    coarse_probs = np.asarray(coarse_probs, np.float32)
    sigma = np.asarray(sigma, np.float32)
    w_feat = np.asarray(w_feat, np.float32)
    w_fuse = np.asarray(w_fuse, np.float32)
    bn_gamma = np.asarray(bn_gamma, np.float32)
    bn_beta = np.asarray(bn_beta, np.float32)
    bn_mean = np.asarray(bn_mean, np.float32)
    bn_var = np.asarray(bn_var, np.float32)
    in_maps = _host_prep(x, coarse_probs, sigma, w_feat, w_fuse, bn_gamma,
                         bn_beta, bn_mean, bn_var)
    try:
        return _device_run(in_maps)
    except Exception as e:
        import sys, traceback
        traceback.print_exc()
        print(f"kernel: device path failed ({type(e).__name__}); "
              f"using host fallback", file=sys.stderr)
        return _host_fallback(x, coarse_probs, sigma, w_feat, w_fuse,
                              bn_gamma, bn_beta, bn_mean, bn_var)
